# revision 66
# baseline (speedup 1.0000x reference)
"""Trainium2 Bass kernel: 4D-CNN ResNet Bottleneck block, SPMD over 8 NeuronCores.

Problem (hardcoded): x[2,256,8,16,16,16] ->
  relu(bn3(conv1x1_256(relu(bn2(conv3x3x3x3(relu(bn1(conv1x1_64(x)))))))) + x)
BatchNorms use training-mode batch stats over (B,T,D,H,W).

Sharding: 16 (b,t) slices -> 2 owned t-slices/core; each core's input slab
includes the +-1 t halo (zero padded at boundaries), so no activation
exchange is needed. Three tiny AllReduces merge the BN statistics.

conv2 structure: the 81 taps of the 3x3x3x3 kernel are PAIRED in the
contraction dim.  Four "class tiles" yA/yB/yC/yD [128, 2*PSL] hold the
padded y1 activations twice: partitions 0-63 unshifted, partitions 64-127
column-shifted by the pair offset (PSL / PH*PW / PW / 1 respectively).  A
single K=128 matmul then computes two taps at once, so each superplane
needs 41 matmuls (27 A + 9 B + 3 C + 1 D-pair + 1 single) instead of 81.
conv2 runs block-OUTER: all 8 PSUM banks accumulate their superplane pair
simultaneously, so conv2 starts as soon as yA is ready (written directly
by the BN1 apply) while the yB/yC/yD fanout DMAs (large 2-dim copies from
the padded-h/w y1rawP staging buffer) complete under the 27 class-A
blocks.  The last blocks run bank-major so the per-bank BN2 stats
pipeline into the conv2 tail.

BN stat AllReduces are issued as early as possible (BN1's right after the
owned superplanes, hidden under halo compute).  conv3's BN stats come
from a Gram matrix instead of a stats pass: C = y2n y2n^T and row-sums s
are built with PE transposes + tiny accumulating matmuls, then
E[y3^2] = diag(W3 C W3^T)/N and mean = W3 s / N, which removes the
DVE-bound bn_stats sweep and most of conv3 pass 1.

Precision: conv1/conv2 run bf16, conv3 runs float32r; the output is
stored fp16 and upcast on the host (|out| <= ~8, well within fp16
range).  The residual is injected into conv3's PSUM by an extra matmul
against diag(1/bn3_scale) with fp16 x, so the epilogue
relu(scale*psum + bias) is a single engine pass per tile.  BN
accumulators and all conv accumulation stay fp32.
"""

import functools
import os

import numpy as np

# ---- problem constants --------------------------------------------------
B, C, T, D, H, W = 2, 256, 8, 16, 16, 16
PL = 64            # bottleneck planes
O3 = 4 * PL        # final channels (256)
NCORES = 8
EPS = 1e-5

TPC = 2                    # owned t-slices per core
SLAB = TPC + 2             # slab slices incl halo
DHW = D * H * W            # 4096
NSP_OWN = TPC * (D // 2)   # 16 owned superplanes (d-pairs)
NPOS_OWN = TPC * DHW       # 8192 positions per core
PW, PH, PD = W + 2, H + 2, D + 2
PHPW = PH * PW             # 324
PSL = PD * PH * PW         # 5832 padded elems per slice
TPSL = 2 * PSL             # class tile width
NBLK = 41                  # conv2 matmul blocks per superplane

MM_DT = os.environ.get("KERNEL_MM_DT", "float32r")  # conv3 matmul dtype
C2_DT = os.environ.get("KERNEL_C2_DT", "bfloat16")  # conv1/conv2 matmul dtype

LAST_RESULT = None  # BassKernelResults of the most recent run (for test.py)


def _tau(kt, kd, kh, kw):
    return kt * 27 + kd * 9 + kh * 3 + kw


def _blocks():
    """conv2 tap pairing: list of (tauA, tauB) per block; block 40 is the
    lone (2,2,2,2) tap (tauB None)."""
    out = []
    for kd in range(3):
        for kh in range(3):
            for kw in range(3):
                out.append((_tau(0, kd, kh, kw), _tau(1, kd, kh, kw)))
    for kh in range(3):
        for kw in range(3):
            out.append((_tau(2, 0, kh, kw), _tau(2, 1, kh, kw)))
    for kw in range(3):
        out.append((_tau(2, 2, 0, kw), _tau(2, 2, 1, kw)))
    out.append((_tau(2, 2, 2, 0), _tau(2, 2, 2, 1)))
    out.append((_tau(2, 2, 2, 2), None))
    return out


@functools.lru_cache(maxsize=4)
def _build(mm_dt_name, c2_dt_name, collectives=True):
    from contextlib import ExitStack

    import concourse.mybir as mybir
    import concourse.tile as tile
    from concourse import bacc

    f32 = mybir.dt.float32
    mmdt = getattr(mybir.dt, mm_dt_name)
    c2dt = getattr(mybir.dt, c2_dt_name)
    AF = mybir.ActivationFunctionType
    AL = mybir.AluOpType

    nc = bacc.Bacc(
        "TRN2",
        target_bir_lowering=False,
        debug=False,
        enable_asserts=False,
        num_devices=NCORES,
    )

    fp16 = mybir.dt.float16
    xsb = nc.dram_tensor("xsb", [2, 128, SLAB * DHW], c2dt,
                         kind="ExternalInput").ap()
    xres = nc.dram_tensor("xres", [2, 128, NPOS_OWN], fp16,
                          kind="ExternalInput").ap()
    idm = nc.dram_tensor("idm", [128, 128], fp16, kind="ExternalInput").ap()
    w1t = nc.dram_tensor("w1t", [128, 4 * PL], c2dt, kind="ExternalInput").ap()
    w2t = nc.dram_tensor("w2t", [128, NBLK * PL], c2dt,
                         kind="ExternalInput").ap()
    w3t = nc.dram_tensor("w3t", [128, O3], f32, kind="ExternalInput").ap()
    gb1 = nc.dram_tensor("gb1", [128, 2], f32, kind="ExternalInput").ap()
    gb2 = nc.dram_tensor("gb2", [128, 2], f32, kind="ExternalInput").ap()
    gb3 = nc.dram_tensor("gb3", [128, 4], f32, kind="ExternalInput").ap()
    tmask = nc.dram_tensor("tmask", [128, SLAB], f32, kind="ExternalInput").ap()
    out = nc.dram_tensor("out", [2, 128, NPOS_OWN], fp16,
                         kind="ExternalOutput").ap()

    cc1_in = nc.dram_tensor("cc1_in", [128, 2], f32).ap()
    cc1_out = nc.dram_tensor("cc1_out", [128, 2], f32,
                             addr_space="Shared").ap()
    cc2_in = nc.dram_tensor("cc2_in", [128, 4], f32).ap()
    cc2_out = nc.dram_tensor("cc2_out", [128, 4], f32, addr_space="Shared").ap()
    cc3_in = nc.dram_tensor("cc3_in", [128, 4], f32).ap()
    cc3_out = nc.dram_tensor("cc3_out", [128, 4], f32, addr_space="Shared").ap()
    RG = [list(range(NCORES))]

    def allreduce(cin, cout):
        if collectives:
            nc.gpsimd.collective_compute(
                "AllReduce", AL.add, replica_groups=RG,
                ins=[cin], outs=[cout],
            )
        else:  # timing-sim variant: stand-in DMA with the same deps
            nc.sync.dma_start(out=cout, in_=cin)

    with tile.TileContext(nc) as tc, ExitStack() as st:
        const = st.enter_context(tc.tile_pool(name="const", bufs=1))
        smalls = st.enter_context(tc.tile_pool(name="smalls", bufs=1))

        def sm(shape, nm):
            return smalls.tile(shape, f32, tag=nm, name=nm)

        # ---- persistent SBUF tensors ---------------------------------
        stg = st.enter_context(tc.tile_pool(name="stg", bufs=1))
        # w1sb holds each 128-row K block as [K,128] with the 64 output
        # channels DUPLICATED in the M dim: conv1 writes identical copies to
        # psum partitions 0-63 / 64-127 so both class-tile halves can be fed
        # from the matching y1raw half with no cross-partition moves.
        w1sb = const.tile([128, 4 * PL], c2dt, tag="w1sb", name="w1sb")
        w2sb = const.tile([128, NBLK * PL], c2dt, tag="w2sb", name="w2sb")
        w3sb = const.tile([128, O3], mmdt, tag="w3sb", name="w3sb")

        def load_round(dst_mm, src_dram, ncols):
            done = 0
            while done < ncols:
                n = min(1024, ncols - done)
                t = stg.tile([128, 1024], f32, tag="stg")
                nc.sync.dma_start(out=t[:, :n], in_=src_dram[:, done:done + n])
                nc.vector.tensor_copy(out=dst_mm[:, done:done + n],
                                      in_=t[:, :n])
                done += n
        gb1sb = sm([128, 2], "gb1sb")
        gb2sb = sm([128, 2], "gb2sb")
        gb3sb = sm([128, 4], "gb3sb")
        tmsb = sm([128, SLAB], "tmsb")
        epsb = sm([128, 1], "epsb")
        nc.vector.memset(epsb[:], EPS)
        y2 = const.tile([128, NSP_OWN * 256], c2dt, tag="y2", name="y2")
        st1 = sm([128, NSP_OWN * 6], "st1")
        st2 = sm([128, 8 * 6], "st2")

        idmsb = const.tile([128, 128], fp16, tag="idmsb", name="idmsb")
        diag3 = const.tile([128, 256], fp16, tag="diag3", name="diag3")
        idf32 = const.tile([128, 128], mmdt, tag="idf32", name="idf32")
        onesb = const.tile([128, 1], c2dt, tag="onesb", name="onesb")
        w3bf = const.tile([64, 256], c2dt, tag="w3bf", name="w3bf")
        # only what conv1 needs up front; the rest loads after the x
        # slices are in flight (DMA device is serialized)
        nc.sync.dma_start(out=w1sb[:], in_=w1t[:])

        def load_rest():
            nc.sync.dma_start(out=w2sb[:], in_=w2t[:])
            nc.sync.dma_start(out=idmsb[:], in_=idm[:])
            load_round(w3sb, w3t, O3)
            nc.vector.tensor_copy(out=idf32[:], in_=idmsb[:])
            nc.vector.memset(onesb[:], 1.0)
            nc.vector.tensor_copy(out=w3bf[:],
                                  in_=w3sb[0:64, :].bitcast(f32))
            nc.sync.dma_start(out=gb1sb[:], in_=gb1[:])
            nc.sync.dma_start(out=gb2sb[:], in_=gb2[:])
            nc.sync.dma_start(out=gb3sb[:], in_=gb3[:])
            nc.sync.dma_start(out=tmsb[:], in_=tmask[:])

        # ---- BN finalize helpers -------------------------------------
        def bn_reduce_prep(mv, arin):
            """arin[:,0]=local mean, arin[:,1]=local E[x^2]."""
            nc.vector.tensor_scalar(arin[:, 1:2], mv[:, 0:1], mv[:, 0:1],
                                    mv[:, 1:2], op0=AL.mult, op1=AL.add)
            nc.vector.tensor_copy(out=arin[:, 0:1], in_=mv[:, 0:1])

        def bn_finalize(sums, inv_n, g_ap, b_ap, scale, bias, nm):
            """sums[:,0]=sum(mean_l), sums[:,1]=sum(e2_l) -> scale/bias."""
            P = sums.shape[0]
            me = sm([P, 2], f"me_{nm}")
            tt = sm([P, 1], f"tt_{nm}")
            rstd = sm([P, 1], f"rstd_{nm}")
            nc.vector.tensor_scalar_mul(me[:], sums[:, 0:2], inv_n)
            nc.vector.tensor_tensor(out=tt[:], in0=me[:, 0:1], in1=me[:, 0:1],
                                    op=AL.mult)
            nc.vector.tensor_tensor(out=tt[:], in0=me[:, 1:2], in1=tt[:],
                                    op=AL.subtract)
            nc.scalar.activation(rstd[:], tt[:], AF.Sqrt, bias=epsb[:],
                                 scale=1.0)
            nc.vector.reciprocal(out=rstd[:], in_=rstd[:])
            nc.vector.tensor_tensor(out=scale[:], in0=g_ap, in1=rstd[:],
                                    op=AL.mult)
            nc.vector.tensor_tensor(out=tt[:], in0=me[:, 0:1], in1=scale[:],
                                    op=AL.mult)
            nc.vector.tensor_tensor(out=bias[:], in0=b_ap, in1=tt[:],
                                    op=AL.subtract)

        scale1 = sm([128, 1], "scale1")
        bias1 = sm([128, 1], "bias1")
        scale2 = sm([128, 1], "scale2")
        bias2 = sm([128, 1], "bias2")
        scale3 = sm([128, 2], "scale3")
        bias3 = sm([128, 2], "bias3")

        with tc.tile_pool(name="ypool", bufs=1) as ypool:
            # class tiles: lower half unshifted, upper half shifted
            yts = {}
            for nm in ("yA", "yB", "yC", "yD"):
                yts[nm] = ypool.tile([128, TPSL], c2dt, tag=nm, name=nm)
            y5 = {nm: t[:].rearrange("p (sl d h w) -> p sl d h w",
                                     sl=2, d=PD, h=PH, w=PW)
                  for nm, t in yts.items()}
            y1p_cm = tc.tile_pool(name="y1p", bufs=1)
            y1p = y1p_cm.__enter__()
            # y1rawP: conv1 output staged bf16 in padded-h/w layout:
            # 4 slice blocks (slab order s=0..3) x 16 d-planes x 18x18 h/w
            # (borders pre-zeroed).  The class-tile fanout is then a few
            # large 2-dim DMA copies (DMA APs allow at most 2 free dims).
            SLW = 16 * PHPW  # 5184 cols per slice block
            y1rawP = y1p.tile([128, 4 * SLW], c2dt, tag="y1rawP",
                              name="y1rawP")
            y1r5 = y1rawP[:].rearrange("p (b d h w) -> p b d h w",
                                       b=4, d=16, h=PH, w=PW)
            zsc = y1p.tile([128, 1296], c2dt, tag="zsc", name="zsc")
            nc.gpsimd.memset(zsc[:], 0.0)

            def zcopy(dst, a, b, c2, plo=0, phi=128):
                src = zsc[plo:phi, :a * b * c2].rearrange(
                    "p (x y z) -> p x y z", x=a, y=b, z=c2)
                nc.vector.tensor_copy(out=dst, in_=src)

            # h/w borders of y1rawP (copied into the class tiles as-is)
            for blk in range(4):
                zcopy(y1r5[:, blk, :, 0:PH:PH - 1, :], 16, 2, PW)
                zcopy(y1r5[:, blk, :, 1:17, 0:PW:PW - 1], 16, 16, 2)

            # yA is written directly by the BN1 apply (strided interior
            # writes), so its h/w borders need zeroing too
            for plo, phi in ((0, 64), (64, 128)):
                for sl in range(2):
                    zcopy(y5["yA"][plo:phi, sl, 1:17, 0:PH:PH - 1, :],
                          16, 2, PW, plo, phi)
                    zcopy(y5["yA"][plo:phi, sl, 1:17, 1:17, 0:PW:PW - 1],
                          16, 16, 2, plo, phi)

            # unwritten d-planes of the class tiles (the fanout writes
            # d rows 1..16, or 0..15 for the PHPW-shifted yB upper half)
            for nm in ("yA", "yB", "yC", "yD"):
                zcopy(y5[nm][0:64, :, 0:PD:PD - 1].rearrange(
                    "p sl d h w -> p sl d (h w)"), 2, 2, PHPW, 0, 64)
            for nm, dsel in (("yA", (0, PD - 1)), ("yB", (16, 17)),
                             ("yC", (0, PD - 1)), ("yD", (0, PD - 1))):
                if dsel == (0, PD - 1):
                    v = y5[nm][64:128, :, 0:PD:PD - 1]
                else:
                    v = y5[nm][64:128, :, 16:18]
                zcopy(v.rearrange("p sl d h w -> p sl d (h w)"),
                      2, 2, PHPW, 64, 128)

            # ======== conv1 (single pass) =============================
            # Streams x, computes conv1 into PSUM (output channels
            # duplicated across both partition halves via w1sb's M-dup),
            # stages the raw result to y1raw (bf16).  Owned superplanes run
            # first so the BN1 stats AllReduce is issued as early as
            # possible; halo superplanes and staging overlap its latency.
            sp_order = ([(s, dp) for s in (1, 2) for dp in range(D // 2)]
                        + [(s, dp) for s in (0, 3) for dp in range(D // 2)])
            with tc.tile_pool(name="ps2", bufs=6, space="PSUM") as ps2, \
                 tc.tile_pool(name="xh", bufs=5) as xh:
                # whole-slice input loads (few big DMAs: HWDGE serializes
                # at ~0.6us per DMA, so batching matters)
                xsl = {}

                def slice_tile(cb, s):
                    if (cb, s) not in xsl:
                        t = xh.tile([128, DHW], c2dt, tag="xslc")
                        if s == 1:  # chunked: the first matmuls start sooner
                            for q in range(2):
                                nc.sync.dma_start(
                                    out=t[:, q * 2048:(q + 1) * 2048],
                                    in_=xsb[cb, :, s * DHW + q * 2048:
                                            s * DHW + (q + 1) * 2048])
                        else:
                            nc.sync.dma_start(
                                out=t[:],
                                in_=xsb[cb, :, s * DHW:(s + 1) * DHW])
                        xsl[(cb, s)] = t
                    return xsl[(cb, s)]

                for s in (1, 2, 0, 3):  # owned slices prefetch first
                    for cb in range(2):
                        slice_tile(cb, s)
                load_rest()

                def c1_sp(u, s, dp, stage_eng):
                    ps = ps2.tile([128, 512], f32, tag="c1p2")
                    for cb in range(2):
                        nc.tensor.matmul(
                            ps[:],
                            lhsT=w1sb[:, cb * 2 * PL:(cb + 1) * 2 * PL],
                            rhs=slice_tile(cb, s)[:, dp * 512:(dp + 1) * 512],
                            start=(cb == 0), stop=(cb == 1),
                        )
                    if u < 16:
                        nc.vector.bn_stats(out=st1[:, u * 6:(u + 1) * 6],
                                           in_=ps[:])
                    dst = y1r5[:, s, 2 * dp:2 * dp + 2, 1:17, 1:17]
                    src = ps[:].rearrange("p (d h w) -> p d h w",
                                          d=2, h=16, w=16)
                    if stage_eng == "dve":
                        nc.vector.tensor_copy(out=dst, in_=src)
                    else:
                        nc.scalar.copy(out=dst, in_=src)

                # owned superplanes first: stats on DVE, staging on ACT
                for u, (s, dp) in enumerate(sp_order[:16]):
                    c1_sp(u, s, dp, "act")

                # the BN1 AllReduce is issued NOW so its round trip hides
                # under the halo superplane compute below
                mv1 = sm([128, 2], "mv1")
                arin1 = sm([128, 2], "arin1")
                nc.vector.bn_aggr(out=mv1[:], in_=st1[:])
                bn_reduce_prep(mv1, arin1)
                nc.sync.dma_start(out=cc1_in[:], in_=arin1[:])
                allreduce(cc1_in[:], cc1_out[:])

                for u, (s, dp) in enumerate(sp_order[16:]):
                    c1_sp(16 + u, s, dp, "act")

                g1s = sm([128, 2], "g1s")
                nc.sync.dma_start(out=g1s[:], in_=cc1_out[:])
                bn_finalize(g1s, 1.0 / NCORES, gb1sb[:, 0:1], gb1sb[:, 1:2],
                            scale1, bias1, "bn1")
                def mk_masked(s):
                    scs = sm([128, 1], f"scale1_s{s}")
                    bis = sm([128, 1], f"bias1_s{s}")
                    nc.vector.tensor_tensor(out=scs[:], in0=scale1[:],
                                            in1=tmsb[:, s:s + 1], op=AL.mult)
                    nc.vector.tensor_tensor(out=bis[:], in0=bias1[:],
                                            in1=tmsb[:, s:s + 1], op=AL.mult)
                    return scs, bis

                # fused BN1 + relu.  yA's four interior regions are
                # written DIRECTLY (no fanout DMA on the critical path);
                # slices 2/3 are additionally applied in place on y1rawP
                # as the source for the yB/yC/yD fanout copies.
                def yA_dst(plo, phi, sl):
                    return y5["yA"][plo:phi, sl, 1:17, 1:17, 1:17]

                # DVE: s2 -> yA upper sl1 (issued first)
                d_ = yA_dst(64, 128, 1)
                nc.vector.tensor_scalar(d_, y1r5[64:128, 2, :, 1:17, 1:17],
                                        scale1[64:128, :], bias1[64:128, :],
                                        op0=AL.mult, op1=AL.add)
                nc.vector.tensor_scalar_max(d_, d_, 0.0)
                # ACT: s1 -> yA lower sl1 and upper sl0 (fused relu)
                nc.scalar.activation(yA_dst(0, 64, 1),
                                     y1r5[0:64, 1, :, 1:17, 1:17],
                                     AF.Relu, bias=bias1[0:64, :], scale=scale1[0:64, :])
                nc.scalar.activation(yA_dst(64, 128, 0),
                                     y1r5[64:128, 1, :, 1:17, 1:17],
                                     AF.Relu, bias=bias1[64:128, :],
                                     scale=scale1[64:128, :])
                sc_h, bi_h = {}, {}
                sc_h[0], bi_h[0] = mk_masked(0)
                sc_h[3], bi_h[3] = mk_masked(3)
                d_ = yA_dst(0, 64, 0)
                nc.vector.tensor_scalar(d_, y1r5[0:64, 0, :, 1:17, 1:17],
                                        sc_h[0][0:64, :], bi_h[0][0:64, :],
                                        op0=AL.mult, op1=AL.add)
                nc.vector.tensor_scalar_max(d_, d_, 0.0)
                # in-place s2 (DVE) and s3 (ACT) for the B/C/D fanout
                iv = y1r5[:, 2, :, 1:17, 1:17]
                nc.vector.tensor_scalar(iv, iv, scale1[:], bias1[:],
                                        op0=AL.mult, op1=AL.add)
                nc.vector.tensor_scalar_max(iv, iv, 0.0)
                iv = y1r5[:, 3, :, 1:17, 1:17]
                nc.scalar.activation(iv, iv, AF.Relu,
                                     bias=bi_h[3][:], scale=sc_h[3][:])

            # ---- fanout: 10 large DMA copies into the class tiles ------
            # src slice block s occupies y1rawP cols [s*SLW, (s+1)*SLW);
            # both partition halves of y1rawP hold identical data.
            def fan(nm, plo, phi, s0, da, trim=0):
                # copies 2 slice blocks (s0, s0+1) into slices 0,1 of the
                # class tile half, d rows da..da+16, hw shifted by -trim
                if trim == 0:
                    dst = y5[nm][plo:phi, :, da:da + 16]
                    dst = dst.rearrange("p sl d h w -> p sl (d h w)")
                    src = y1rawP[plo:phi, s0 * SLW:(s0 + 2) * SLW].rearrange(
                        "p (sl c) -> p sl c", sl=2)
                    nc.gpsimd.dma_start(out=dst, in_=src)
                else:
                    for sl in range(2):
                        dst = y5[nm][plo:phi, sl, da:da + 16]
                        dst = dst.rearrange("p d h w -> p (d h w)")
                        dst = dst[:, 0:16 * PHPW - trim]
                        src = y1rawP[plo:phi, (s0 + sl) * SLW + trim:
                                     (s0 + sl + 1) * SLW]
                        nc.gpsimd.dma_start(out=dst, in_=src)

            fan("yB", 0, 64, 2, 1)        # slices 2,3
            fan("yB", 64, 128, 2, 0)      # shift PHPW via d-row offset
            fan("yC", 0, 64, 2, 1)
            fan("yC", 64, 128, 2, 1, trim=PW)   # shift PW
            fan("yD", 0, 64, 2, 1)
            fan("yD", 64, 128, 2, 1, trim=1)    # shift 1

            y1p_cm.__exit__(None, None, None)  # frees y1rawP/zsc

            # ======== conv2: 81-tap 4D conv via 41 paired matmuls ======
            blocks = _blocks()

            def rhs_view(bi, sp):
                s = 1 + sp // 8
                dp = sp % 8
                ta, tb = blocks[bi]
                kt, r = divmod(ta, 27)
                kd, r2 = divmod(r, 9)
                kh, kw = divmod(r2, 3)
                if bi < 27:          # class A: d0 = 2dp+kd, windows kh,kw
                    return y5["yA"][:, s - 1, 2 * dp + kd:2 * dp + kd + 2,
                                    kh:kh + 16, kw:kw + 16]
                if bi < 36:          # class B: kt=2, kd base 0
                    return y5["yB"][:, s - 1, 2 * dp:2 * dp + 2,
                                    kh:kh + 16, kw:kw + 16]
                if bi < 39:          # class C: kt=2, kd=2, kh base 0
                    return y5["yC"][:, s - 1, 2 * dp + 2:2 * dp + 4,
                                    0:16, kw:kw + 16]
                if bi == 39:         # class D pair: kw base 0
                    return y5["yD"][:, s - 1, 2 * dp + 2:2 * dp + 4,
                                    2:18, 0:16]
                # single (2,2,2,2) on the lower (unshifted) half
                return y5["yD"][0:64, s - 1, 2 * dp + 2:2 * dp + 4,
                                2:18, 2:18]

            # block-outer: all 8 PSUM banks accumulate their superplane
            # pair simultaneously; class-A blocks (27 of 41) only need yA,
            # so conv2 starts while the yB/yC/yD fanout DMAs are in flight.
            with tc.tile_pool(name="ps3", bufs=8, space="PSUM") as ps3:
                pss = [ps3.tile([128, 512], f32, tag="c2p", name=f"c2p{k}")
                       for k in range(8)]
                TAIL = 38  # last blocks run bank-major so banks finish
                           # staggered and their stats pipeline

                def c2mm(bi, k):
                    single = blocks[bi][1] is None
                    lh = (w2sb[0:64, bi * 64:(bi + 1) * 64] if single
                          else w2sb[:, bi * 64:(bi + 1) * 64])
                    for half, sp in ((0, 2 * k), (1, 2 * k + 1)):
                        nc.tensor.matmul(
                            pss[k][half * 64:(half + 1) * 64, :],
                            lhsT=lh, rhs=rhs_view(bi, sp),
                            start=(bi == 0), stop=(bi == NBLK - 1))

                for bi in range(TAIL):
                    for k in range(8):
                        c2mm(bi, k)
                for k in range(8):
                    for bi in range(TAIL, NBLK):
                        c2mm(bi, k)
                    nc.vector.bn_stats(out=st2[:, k * 6:(k + 1) * 6],
                                       in_=pss[k][:])
                    nc.scalar.copy(out=y2[:, k * 512:(k + 1) * 512],
                                   in_=pss[k][:])

        # y-tiles freed here
        mv2 = sm([128, 2], "mv2")
        arin2 = sm([128, 2], "arin2")
        nc.vector.bn_aggr(out=mv2[:], in_=st2[:])
        bn_reduce_prep(mv2, arin2)
        # cc2_in cols 0-1 = local stats, cols 2-3 = the same with the
        # partition halves swapped, so no post-AllReduce swap DMA is needed
        nc.sync.dma_start(out=cc2_in[:, 0:2], in_=arin2[:])
        nc.sync.dma_start(out=cc2_in[0:64, 2:4], in_=arin2[64:128, :])
        nc.sync.dma_start(out=cc2_in[64:128, 2:4], in_=arin2[0:64, :])
        allreduce(cc2_in[:], cc2_out[:])
        fa = sm([128, 4], "fa2")
        nc.sync.dma_start(out=fa[:], in_=cc2_out[:])
        nc.vector.tensor_tensor(out=fa[:, 0:2], in0=fa[:, 0:2],
                                in1=fa[:, 2:4], op=AL.add)
        bn_finalize(fa[:, 0:2], 1.0 / (2 * NCORES), gb2sb[:, 0:1],
                    gb2sb[:, 1:2], scale2, bias2, "bn2")

        with tc.tile_pool(name="zp", bufs=1) as zpool:
            xrt = [zpool.tile([128, NPOS_OWN], fp16, tag=f"xr{oh}",
                              name=f"xr{oh}") for oh in range(2)]

            def get_xt9(idx):
                sp, oh = divmod(idx, 2)
                return xrt[oh][:, sp * 512:(sp + 1) * 512]

            y2n = zpool.tile([128, NSP_OWN * 256], mmdt, tag="y2n", name="y2n")
            for k in range(8):  # BN2 + relu (rounds to matmul dtype)
                yk = y2n[:, k * 512:(k + 1) * 512]
                if k % 2 == 0:
                    nc.scalar.activation(
                        yk, y2[:, k * 512:(k + 1) * 512],
                        AF.Relu, bias=bias2[:], scale=scale2[:])
                else:
                    nc.vector.tensor_scalar(
                        yk, y2[:, k * 512:(k + 1) * 512],
                        scale2[:], bias2[:], op0=AL.mult, op1=AL.add)
                    nc.vector.tensor_scalar_max(yk, yk, 0.0)
            # residual-x loads: the tiny head-writes depend on the BN2
            # result, so the big transfers (WAW on the heads) cannot hog
            # the DMA device while the AR2 chain runs; they then overlap
            # the conv3 stats phase
            for oh in range(2):
                for c4 in range(4):
                    nc.vector.tensor_copy(
                        out=xrt[oh][:, c4 * 2048:c4 * 2048 + 1],
                        in_=scale2[:])
                    nc.scalar.dma_start(
                        out=xrt[oh][:, c4 * 2048:(c4 + 1) * 2048],
                        in_=xres[oh, :, c4 * 2048:(c4 + 1) * 2048])

            def c3_mm(ps4, k, half, oh, stop=True):
                rhs = y2n[half * 64:(half + 1) * 64,
                          k * 512:(k + 1) * 512].bitcast(mmdt)
                pg = ps4.tile([128, 512], f32, tag="c3")
                nc.tensor.matmul(
                    pg[:],
                    lhsT=w3sb[half * 64:(half + 1) * 64,
                              oh * 128:(oh + 1) * 128].bitcast(mmdt),
                    rhs=rhs, start=True, stop=stop)
                return pg

            # ======== conv3 stats via Gram matrix ======================
            # y3 = W3 y2n, so mean(y3) = W3 s / N and E[y3^2] =
            # diag(W3 C W3^T) / N with s = row-sums and C = y2n y2n^T.
            # C and s come from PE transposes + tiny accumulating matmuls
            # instead of a full stats pass (which was DVE-bound).
            arin3 = sm([128, 4], "arin3")
            s_f32 = sm([1, 64], "s_f32")
            sT32 = sm([64, 1], "sT32")
            m3s = sm([1, 256], "m3s")
            e3s = sm([1, 256], "e3s")
            with tc.tile_pool(name="ps4", bufs=1, space="PSUM") as ps4, \
                 tc.tile_pool(name="ps4t", bufs=3, space="PSUM") as ps4t, \
                 tc.tile_pool(name="tsp", bufs=4) as tsp:
                Cps = ps4.tile([64, 64], f32, tag="Cps", name="Cps")
                sps = ps4.tile([1, 64], f32, tag="sps", name="sps")
                for g in range(8):
                    tp = ps4t.tile([128, 512], f32, tag="tp")
                    for j in range(4):
                        ch = g * 4 + j
                        nc.tensor.transpose(
                            tp[:, j * 128:(j + 1) * 128].bitcast(mmdt),
                            y2n[:, ch * 128:(ch + 1) * 128].bitcast(mmdt),
                            idf32[:])
                    tsb = tsp.tile([128, 512], c2dt, tag="tsb")
                    if g % 3 != 2:
                        nc.scalar.copy(out=tsb[:], in_=tp[:])
                    else:
                        nc.vector.tensor_copy(out=tsb[:], in_=tp[:])
                    for j in range(4):
                        for hh in range(2):
                            cv = tsb[:, j * 128 + hh * 64:
                                     j * 128 + (hh + 1) * 64]
                            first = (g == 0 and j == 0 and hh == 0)
                            last = (g == 7 and j == 3 and hh == 1)
                            nc.tensor.matmul(Cps[:], lhsT=cv, rhs=cv,
                                             start=first, stop=last)
                            nc.tensor.matmul(sps[:], lhsT=onesb[:, 0:1],
                                             rhs=cv, start=first, stop=last)
                # s -> [64,1] bf16 (via f32 round trip and a scatter DMA)
                nc.vector.tensor_copy(out=s_f32[:], in_=sps[:])
                nc.sync.dma_start(out=sT32[:], in_=s_f32[:])
                sTb = tsp.tile([64, 1], c2dt, tag="sTb", name="sTb")
                nc.vector.tensor_copy(out=sTb[:], in_=sT32[:])
                Csb = tsp.tile([64, 64], c2dt, tag="Csb", name="Csb")
                nc.vector.tensor_copy(out=Csb[:], in_=Cps[:])
                T1 = ps4.tile([64, 256], f32, tag="T1", name="T1")
                nc.tensor.matmul(T1[:], lhsT=Csb[:], rhs=w3bf[:],
                                 start=True, stop=True)
                T1sb = tsp.tile([64, 256], c2dt, tag="T1sb", name="T1sb")
                nc.vector.tensor_copy(out=T1sb[:], in_=T1[:])
                Ee = tsp.tile([64, 256], c2dt, tag="Ee", name="Ee")
                nc.vector.tensor_tensor(out=Ee[:], in0=T1sb[:], in1=w3bf[:],
                                        op=AL.mult)
                e3p = ps4.tile([1, 256], f32, tag="e3p", name="e3p")
                nc.tensor.matmul(e3p[:], lhsT=onesb[0:64, 0:1], rhs=Ee[:],
                                 start=True, stop=True)
                m3p = ps4.tile([1, 256], f32, tag="m3p", name="m3p")
                nc.tensor.matmul(m3p[:], lhsT=sTb[:], rhs=w3bf[:],
                                 start=True, stop=True)
                inv_n = 1.0 / NPOS_OWN
                nc.vector.tensor_scalar_mul(m3s[:], m3p[:], inv_n)
                nc.vector.tensor_scalar_mul(e3s[:], e3p[:], inv_n)
                # dst traversal must be oh-major to match the row layout
                ar4 = cc3_in.rearrange("p (oh j) -> oh p j", oh=2, j=2)
                nc.sync.dma_start(out=ar4[:, :, 0], in_=m3s[:])
                nc.sync.dma_start(out=ar4[:, :, 1], in_=e3s[:])
            allreduce(cc3_in[:], cc3_out[:])
            g3s = sm([128, 4], "g3s")
            nc.sync.dma_start(out=g3s[:], in_=cc3_out[:])
            recip3 = sm([128, 2], "recip3")
            for oh in range(2):
                bn_finalize(g3s[:, oh * 2:oh * 2 + 2], 1.0 / NCORES,
                            gb3sb[:, oh:oh + 1], gb3sb[:, 2 + oh:3 + oh],
                            scale3[:, oh:oh + 1], bias3[:, oh:oh + 1],
                            f"bn3_{oh}")
            nc.vector.reciprocal(out=recip3[:], in_=scale3[:])
            for oh in range(2):
                # diag(1/scale3): lets the PE inject the residual into PSUM
                nc.vector.tensor_scalar_mul(
                    diag3[:, oh * 128:(oh + 1) * 128], idmsb[:],
                    recip3[:, oh:oh + 1])

            # ==== conv3 pass 2 + fused BN3/residual/relu/store ========
            with tc.tile_pool(name="ps5", bufs=8, space="PSUM") as ps5, \
                 tc.tile_pool(name="fino", bufs=6) as fino:
                o9 = {}
                for k in range(8):
                    P, Q = 2 * k, 2 * k + 1
                    for half, sp in ((0, P), (1, Q)):
                        for oh in range(2):
                            idx = sp * 2 + oh
                            pg = c3_mm(ps5, k, half, oh, stop=False)
                            nc.tensor.matmul(
                                pg[:], lhsT=diag3[:, oh * 128:(oh + 1) * 128],
                                rhs=get_xt9(idx), start=False, stop=True)
                            o9[oh] = fino.tile([128, 512], fp16,
                                               tag="o9", name="o9")
                            j = 0
                            dst = o9[oh][:, j * 512:(j + 1) * 512]
                            if idx % 3 == 2:  # spread epilogue over DVE too
                                nc.vector.tensor_scalar(
                                    dst, pg[:], scale3[:, oh:oh + 1],
                                    bias3[:, oh:oh + 1],
                                    op0=AL.mult, op1=AL.add)
                                nc.vector.tensor_scalar_max(dst, dst, 0.0)
                            else:
                                nc.scalar.activation(
                                    dst, pg[:], AF.Relu,
                                    bias=bias3[:, oh:oh + 1],
                                    scale=scale3[:, oh:oh + 1])
                            nc.sync.dma_start(
                                out=out[oh, :, sp * 512:(sp + 1) * 512],
                                in_=o9[oh][:])

    nc.compile()
    return nc


# ---- host-side input prep / output assembly -----------------------------

def _prep_inputs(x, w1, g1, b1, w2, g2, b2, w3, g3, b3):
    import ml_dtypes
    f4 = np.float32
    bf = ml_dtypes.bfloat16
    xr = np.ascontiguousarray(x, f4).reshape(B, C, T, DHW)

    w2r = np.ascontiguousarray(w2, f4).reshape(PL, PL, 81)
    A = w2r.transpose(2, 1, 0)  # [81, c, o]
    w2t = np.zeros((128, NBLK * PL), f4)
    for j, (ta, tb) in enumerate(_blocks()):
        w2t[0:64, j * PL:(j + 1) * PL] = A[ta]
        if tb is not None:
            w2t[64:128, j * PL:(j + 1) * PL] = A[tb]
    w2t = w2t.astype(bf)

    w1T = np.ascontiguousarray(w1, f4).T.reshape(2, 128, PL)  # [cb, k, o]
    w1t = np.ascontiguousarray(
        np.concatenate([np.concatenate([w1T[cb]] * 2, 1) for cb in range(2)], 1)
    ).astype(bf)
    w3t = np.concatenate([np.ascontiguousarray(w3, f4).T] * 2, 0).copy()

    gb1 = np.stack([np.asarray(g1, f4), np.asarray(b1, f4)], 1)
    gb1 = np.concatenate([gb1, gb1], 0)
    gb2 = np.stack([np.asarray(g2, f4), np.asarray(b2, f4)], 1)
    gb2 = np.concatenate([gb2, gb2], 0)
    g3r = np.asarray(g3, f4).reshape(2, 128).T
    b3r = np.asarray(b3, f4).reshape(2, 128).T
    gb3 = np.concatenate([g3r, b3r], 1).copy()  # [128,4]

    in_maps = []
    for core in range(NCORES):
        b = core // 4
        t0 = 2 * (core % 4)
        xslab = np.zeros((C, SLAB, DHW), f4)
        tm = np.zeros((SLAB,), f4)
        for si, gt in enumerate(range(t0 - 1, t0 + 3)):
            if 0 <= gt < T:
                xslab[:, si] = xr[b, :, gt]
                tm[si] = 1.0
        xs2 = xslab.reshape(2, 128, SLAB * DHW)
        in_maps.append({
            "xsb": np.ascontiguousarray(xs2).astype(bf),
            "xres": np.ascontiguousarray(
                xs2[:, :, DHW:DHW + NPOS_OWN]).astype(np.float16),
            "idm": np.eye(128, dtype=np.float16),
            "w1t": w1t, "w2t": w2t, "w3t": w3t,
            "gb1": gb1, "gb2": gb2, "gb3": gb3,
            "tmask": np.broadcast_to(tm, (128, SLAB)).copy(),
        })
    return in_maps


def kernel(x, w1, g1, b1, w2, g2, b2, w3, g3, b3):
    global LAST_RESULT
    from concourse.bass_utils import run_bass_kernel_spmd

    nc = _build(MM_DT, C2_DT)
    in_maps = _prep_inputs(x, w1, g1, b1, w2, g2, b2, w3, g3, b3)
    res = run_bass_kernel_spmd(nc, in_maps, core_ids=list(range(NCORES)))
    LAST_RESULT = res

    full = np.empty((B, C, T, D, H, W), np.float32)
    for core in range(NCORES):
        b = core // 4
        t0 = 2 * (core % 4)
        o = np.asarray(res.results[core]["out"],
                       np.float32).reshape(C, TPC, D, H, W)
        full[b, :, t0:t0 + TPC] = o
    return full
